# revision 16
# baseline (speedup 1.0000x reference)
import math
import sys

for _p in ("/root/.axon_site", "/root/.axon_site/_ro/trn_rl_repo", "/opt/trn_rl_repo"):
    if _p not in sys.path:
        sys.path.append(_p)

import numpy as np
import ml_dtypes

BF16 = ml_dtypes.bfloat16
F8 = ml_dtypes.float8_e4m3  # IEEE-style e4m3: max 240 == TRN FP8_EXP4

B, L, NS = 8, 1024, 512
D, NH, DN = 512, 8, 256
E, K, HE = 8, 2, 2048
HD = D // NH
EPS = 1e-5
NCORES = 8

SW = 64.0    # attention weight fp8 scale
SI = 64.0    # moe gate fc_in scale
SV = 32.0    # moe val fc_in scale
SO = 64.0    # moe fc_out scale

EA = math.log2(math.e) * 0.125 / 4096.0 * 8.0
EB = 56.0 - 0.344

_NC = None


def _build():
    global _NC
    if _NC is not None:
        return _NC
    from concourse import bass, tile, mybir, masks

    f32 = mybir.dt.float32
    bf16 = mybir.dt.bfloat16
    f8e4 = mybir.dt.float8e4
    u8 = mybir.dt.uint8
    AF = mybir.ActivationFunctionType
    OP = mybir.AluOpType
    DR = mybir.MatmulPerfMode.DoubleRow

    nc = bass.Bass()
    x_h = nc.declare_dram_parameter("x", [L, D], bf16, isOutput=False)
    scene_h = nc.declare_dram_parameter("scene", [NS, D], bf16, isOutput=False)
    wattn_h = nc.declare_dram_parameter("wattn", [8, D, D], f8e4, isOutput=False)
    battn_h = nc.declare_dram_parameter("battn", [8, D], f32, isOutput=False)
    brow_h = nc.declare_dram_parameter("brow", [1, 1024], bf16, isOutput=False)
    wi_h = nc.declare_dram_parameter("wi", [64, 128, 512], f8e4, isOutput=False)
    bi_h = nc.declare_dram_parameter("bi_t", [64, 128], f32, isOutput=False)
    wo_h = nc.declare_dram_parameter("wo", [16, 128, 1024], f8e4, isOutput=False)
    borow_h = nc.declare_dram_parameter("borow", [1, 512], bf16, isOutput=False)
    out_h = nc.declare_dram_parameter("out", [L, D], f32, isOutput=True)

    with tile.TileContext(nc) as tc, \
         tc.tile_pool(name="sing", bufs=1) as sing, \
         tc.tile_pool(name="p_resid", bufs=2) as p_resid, \
         tc.tile_pool(name="p_xn", bufs=2) as p_xn, \
         tc.tile_pool(name="p_qkv", bufs=1) as p_qkv, \
         tc.tile_pool(name="p_oh", bufs=2) as p_oh, \
         tc.tile_pool(name="p_eT", bufs=4) as p_eT, \
         tc.tile_pool(name="p_ms", bufs=6) as p_ms, \
         tc.tile_pool(name="p_msx", bufs=4) as p_msx, \
         tc.tile_pool(name="p_xb", bufs=2) as p_xb, \
         tc.tile_pool(name="p_wo", bufs=4) as p_wo, \
         tc.tile_pool(name="p_orm", bufs=2) as p_orm, \
         tc.tile_pool(name="p_nt", bufs=2) as p_nt, \
         tc.tile_pool(name="psA", bufs=2, space="PSUM") as psA, \
         tc.tile_pool(name="psB", bufs=2, space="PSUM") as psB:

        dma_s = nc.sync.dma_start
        dma_a = nc.scalar.dma_start

        ident = sing.tile([128, 128], f32, name="ident", tag="ident")
        masks.make_identity(nc, ident[:])
        ones_sq = sing.tile([128, 64], bf16, name="ones_sq", tag="ones_sq")
        nc.vector.memset(ones_sq[:], 1.0)
        ones_q = sing.tile([1, 512], bf16, name="ones_q", tag="ones_qq")
        nc.vector.memset(ones_q[:], 1.0)
        ones_big = sing.tile([128, 512], bf16, name="ones_big", tag="ones_b")
        nc.vector.memset(ones_big[:], 1.0)
        ones_bc = sing.tile([128, 128], bf16, name="ones_bc", tag="ones_bc")
        nc.vector.memset(ones_bc[:], 1.0)

        # HAM warm-up spins + ln/exp table preload during the DMA window
        dummy = p_ms.tile([1, 1], f32, name="dummy", tag="ms")
        nc.scalar.activation(dummy[:], ident[0:1, 0:1], AF.Ln)
        dummy2 = p_ms.tile([1, 1], f32, name="dummy2", tag="ms")
        nc.scalar.activation(dummy2[:], ident[0:1, 0:1], AF.Exp)

        # x -> feature-major bf16 spine directly via DMA crossbar transpose
        X_T = p_resid.tile([128, 4, 1024], bf16, name="X_T", tag="resid")
        for mt in range(4):
            nc.sync.dma_start_transpose(X_T[:, mt, :], x_h[:, mt * 128:(mt + 1) * 128])
        w_attn = sing.tile([128, 8, 4, 512], f8e4, name="w_attn", tag="w_attn")
        dma_s(out=w_attn[:], in_=wattn_h.rearrange("i (t p) d -> p i t d", p=128))
        scene_Tb = p_xb.tile([128, 4, 512], bf16, name="scene_Tb", tag="xbsq")
        for mt in range(4):
            nc.sync.dma_start_transpose(scene_Tb[:, mt, :], scene_h[:, mt * 128:(mt + 1) * 128])
        b_attn = sing.tile([128, 8, 4], f32, name="b_attn", tag="b_attn")
        dma_s(out=b_attn[:], in_=battn_h.rearrange("i (t p) -> p i t", p=128))
        brow = sing.tile([1, 1024], bf16, name="brow", tag="brow")
        dma_s(out=brow[:], in_=brow_h[:, :])
        borow = sing.tile([1, 512], bf16, name="borow", tag="borow")
        dma_s(out=borow[:], in_=borow_h[:, :])
        bi_sb = sing.tile([128, 64], f32, name="bi_sb", tag="bi_sb")
        dma_s(out=bi_sb[:], in_=bi_h.rearrange("b p -> p b"))

        # full moe fc_in weight prefetch (trickles in during CA/SA)
        wi_sb = sing.tile([128, 64, 4, 128], f8e4, name="wi_sb", tag="wi_sb")
        for i in range(64):
            dma_s(out=wi_sb[:, i], in_=wi_h[i, :, :].rearrange("p (t m) -> p t m", t=4))

        for _w in range(12):
            spin = psA.tile([128, 2, 512], f32, name="spin", tag="A")
            nc.tensor.matmul(spin[0:64, 0, :], ones_sq[:, :], ones_big[:],
                             start=True, stop=True)

        scene_T = sing.tile([128, 4, 512], f8e4, name="scene_T", tag="scene_T")
        nc.vector.tensor_scalar_mul(scene_T[:], scene_Tb[:], 1.0)

        # V arena: even heads at 0:64 (+den ones col 64), odd heads at
        # 144:208 (+den ones col 80, zeros 81:144); constant regions set once
        V2 = p_qkv.tile([128, 8, 4, 208], f8e4, name="V2", tag="ve")
        nc.vector.memset(V2[:, :, :, 64:65], 1.0)
        nc.vector.memset(V2[:, :, :, 80:81], 1.0)
        nc.vector.memset(V2[:, :, :, 81:144], 0.0)

        def ln_spins(n):
            for _s in range(n):
                spin = psA.tile([128, 2, 512], f32, name="spin", tag="A")
                nc.tensor.matmul(spin[0:64, 0, :], ones_sq[:, :], ones_big[:],
                                 start=True, stop=True)

        def layer_norm(src, xn):
            ln_spins(3)
            for qc in range(2):
                qs = slice(qc * 512, (qc + 1) * 512)
                sq = p_xb.tile([128, 4, 512], bf16, name="sq", tag="xbsq")
                nc.vector.tensor_tensor(sq[:], src[:, :, qs], src[:, :, qs], OP.mult)
                st = psA.tile([128, 2, 512], f32, name="st", tag="A")
                for kt in range(4):
                    nc.tensor.matmul(st[:, 0, :], ones_bc[:], src[:, kt, qs],
                                     start=(kt == 0), stop=(kt == 3))
                for kt in range(4):
                    nc.tensor.matmul(st[:, 1, :], ones_bc[:], sq[:, kt, :],
                                     start=(kt == 0), stop=(kt == 3))
                m_bf = p_ms.tile([128, 512], bf16, name="m_bf", tag="ms")
                nc.vector.tensor_scalar_mul(m_bf[:], st[:, 0, :], 1.0 / 512.0)
                e2v = p_ms.tile([128, 512], f32, name="e2v", tag="ms")
                nc.vector.tensor_scalar(e2v[:], st[:, 1, :], 1.0 / 512.0, EPS,
                                        OP.mult, OP.add)
                mm = p_ms.tile([128, 512], bf16, name="mm", tag="ms")
                nc.vector.tensor_tensor(mm[:], m_bf[:], m_bf[:], OP.mult)
                var = p_ms.tile([128, 512], f32, name="var", tag="ms")
                nc.vector.tensor_tensor(var[:], e2v[:], mm[:], OP.subtract)
                # 1/sqrt(var) = exp(-0.5 ln var): stays in the ln/exp table set
                lnv = p_ms.tile([128, 512], f32, name="lnv", tag="ms")
                nc.scalar.activation(lnv[:], var[:], AF.Ln)
                rsq = p_ms.tile([128, 512], bf16, name="rsq", tag="ms")
                nc.scalar.activation(rsq[:], lnv[:], AF.Exp, scale=-0.5)
                for mt in range(4):
                    xs = p_msx.tile([128, 512], bf16, name="xs", tag="msx")
                    nc.vector.tensor_tensor(xs[:], src[:, mt, qs], m_bf[:], OP.subtract)
                    nc.vector.tensor_tensor(xn[:, mt, qs], xs[:], rsq[:], OP.mult)
            ln_spins(3)

        def attention(widx, xq_T, kv_T, kv_len, resid_in, resid_out):
            nkp = kv_len // 128
            nkc = kv_len // 512
            nb2 = nkp // 2
            # fp8 DoubleRow projections: psum = 64*W @ x; Scalar drains Q
            Q_T = p_qkv.tile([128, 4, 1024], bf16, name="Q_T", tag="q")
            K_T = p_qkv.tile([128, 4, 1024], bf16, name="K_T", tag="k")
            for mt in range(4):
                q_ps = psA.tile([128, 2, 512], f32, name="q_ps", tag="A")
                for t2 in range(2):
                    for qc in range(2):
                        nc.tensor.matmul(q_ps[:, qc, :],
                                         w_attn[:, widx, 2 * t2:2 * t2 + 2, mt * 128:(mt + 1) * 128],
                                         xq_T[:, 2 * t2:2 * t2 + 2, qc * 512:(qc + 1) * 512],
                                         start=(t2 == 0), stop=(t2 == 1), perf_mode=DR)
                k_ps = psB.tile([128, 2, 512], f32, name="k_ps", tag="B")
                for t2 in range(2):
                    for kc in range(nkc):
                        nc.tensor.matmul(k_ps[:, kc, :],
                                         w_attn[:, widx + 1, 2 * t2:2 * t2 + 2, mt * 128:(mt + 1) * 128],
                                         kv_T[:, 2 * t2:2 * t2 + 2, kc * 512:(kc + 1) * 512],
                                         start=(t2 == 0), stop=(t2 == 1), perf_mode=DR)
                nc.vector.tensor_scalar_add(Q_T[:, mt, :], q_ps[:],
                                            b_attn[:, widx, mt:mt + 1])
                nc.vector.tensor_scalar_add(K_T[:, mt, 0:kv_len],
                                            k_ps[:, 0:nkc, :],
                                            b_attn[:, widx + 1, mt:mt + 1])
            for kp in range(nkp):
                v_ps = psA.tile([128, 2, 4, 64], f32, name="v_ps", tag="A")
                for t2 in range(2):
                    nc.tensor.matmul(v_ps[:],
                                     kv_T[:, 2 * t2:2 * t2 + 2, kp * 128:(kp + 1) * 128],
                                     w_attn[:, widx + 2, 2 * t2:2 * t2 + 2, :],
                                     start=(t2 == 0), stop=(t2 == 1), perf_mode=DR)
                nc.vector.tensor_scalar_mul(V2[:, kp, :, 0:64], v_ps[:, 0, :, :], 1.0)
                nc.vector.tensor_scalar_mul(V2[:, kp, :, 144:208], v_ps[:, 1, :, :], 1.0)

            def emit_norm(st):
                o_tl, rcb, hb, Oh_all, fast = st
                rb = psA.tile([128, 2, 512], f32, name="rb", tag="A")
                nc.tensor.matmul(rb[0:64, 0, :], ones_sq[64:65, 0:64], rcb[64:65, :],
                                 start=True, stop=True)
                nc.tensor.matmul(rb[64:128, 0, :], ones_sq[0:1, 0:64], rcb[0:1, :],
                                 start=True, stop=True)
                rb_sb = p_eT.tile([128, 512], bf16, name="rb_sb", tag="rbs")
                if fast:
                    # rcb already holds 1/den (Scalar path): broadcast only
                    nc.vector.tensor_scalar_mul(rb_sb[:], rb[:, 0, :], 1.0)
                else:
                    # rcb holds den: 1/den via Newton on idle GpSimd
                    den_sb = p_nt.tile([128, 512], f32, name="den_sb", tag="dsb")
                    nc.vector.tensor_scalar_mul(den_sb[:], rb[:, 0, :], 1.0)
                    z0 = 1.0 / float(kv_len)
                    y1 = p_nt.tile([128, 512], f32, name="y1", tag="y1", bufs=1)
                    nc.gpsimd.tensor_scalar(y1[:], den_sb[:], -z0 * z0, 2.0 * z0,
                                            OP.mult, OP.add)
                    tn = p_nt.tile([128, 512], f32, name="tn", tag="tn", bufs=1)
                    nc.gpsimd.tensor_tensor(tn[:], den_sb[:], y1[:], OP.mult)
                    un = p_nt.tile([128, 512], f32, name="un", tag="un", bufs=1)
                    nc.gpsimd.tensor_scalar(un[:], tn[:], -1.0, 2.0, OP.mult, OP.add)
                    nc.gpsimd.tensor_tensor(rb_sb[:], y1[:], un[:], OP.mult)
                nc.vector.tensor_tensor(Oh_all[0:64, hb, :], o_tl[0:64, 0, :],
                                        rb_sb[0:64, :], OP.mult)
                nc.vector.tensor_tensor(Oh_all[64:128, hb, :], o_tl[64:128, 1, :],
                                        rb_sb[64:128, :], OP.mult)

            for qc in range(2):
                qs = slice(qc * 512, (qc + 1) * 512)
                # Oh_all = 64 * attn_out per head, fp8
                Oh_all = p_oh.tile([128, 4, 512], f8e4, name="Oh_all", tag="oh")
                pend = [None]

                def emit_den(o_tl, hb):
                    # denominator rows -> SBUF; queue Oh-normalize one hb late
                    rcb = p_ms.tile([65, 512], bf16, name="rcb", tag="ms")
                    if hb == 3:
                        # last head: no later work hides the Newton chain, so
                        # take the Scalar ln/exp path (Scalar idles here)
                        lnd = p_ms.tile([65, 512], f32, name="lnd", tag="ms")
                        nc.scalar.activation(lnd[64:65, :], o_tl[64:65, 0, :], AF.Ln)
                        nc.scalar.activation(lnd[0:1, :], o_tl[0:1, 1, :], AF.Ln)
                        nc.scalar.activation(rcb[64:65, :], lnd[64:65, :], AF.Exp, scale=-1.0)
                        nc.scalar.activation(rcb[0:1, :], lnd[0:1, :], AF.Exp, scale=-1.0)
                    else:
                        nc.vector.tensor_scalar_mul(rcb[64:65, :], o_tl[64:65, 0, :], 1.0)
                        nc.vector.tensor_scalar_mul(rcb[0:1, :], o_tl[0:1, 1, :], 1.0)
                    pend[0] = (o_tl, rcb, hb, Oh_all, hb == 3)

                def flush_pv(st):
                    # software-pipelined PV: issued after the NEXT step's
                    # scores so the Tensor queue never stalls behind exp
                    o_tl, b2, e2a, e2b_f8, hb = st
                    nc.tensor.matmul(o_tl[0:65, 0, :], V2[:, 2 * b2:2 * b2 + 2, hb, 0:65],
                                     e2a[:], start=(b2 == 0), stop=(b2 == nb2 - 1),
                                     perf_mode=DR)
                    nc.tensor.matmul(o_tl[:, 1, :], V2[:, 2 * b2:2 * b2 + 2, hb, 80:208],
                                     e2b_f8, start=(b2 == 0), stop=(b2 == nb2 - 1),
                                     perf_mode=DR)
                    if b2 == 0 and pend[0] is not None:
                        # previous head's normalize: start its Newton chain now
                        # so it hides under this whole head's steps
                        emit_norm(pend[0])
                        pend[0] = None
                    if b2 == nb2 - 1:
                        emit_den(o_tl, hb)

                pend_pv = None
                for hb in range(4):
                    o_tl = psB.tile([128, 2, 512], f32, name="o_tl", tag="B")
                    for b2 in range(nb2):
                        sa = psA.tile([128, 2, 512], f32, name="sa", tag="A")
                        sb = psA.tile([128, 2, 512], f32, name="sb", tag="A")
                        # row-packed score pairs: both heads concurrently
                        for k2 in range(2):
                            kp = 2 * b2 + k2
                            nc.tensor.matmul(sa[:, k2, :],
                                             K_T[0:64, hb, kp * 128:(kp + 1) * 128],
                                             Q_T[0:64, hb, qs],
                                             start=True, stop=True)
                            nc.tensor.matmul(sb[:, k2, :],
                                             K_T[64:128, hb, kp * 128:(kp + 1) * 128],
                                             Q_T[64:128, hb, qs],
                                             start=True, stop=True)
                        if pend_pv is not None:
                            flush_pv(pend_pv)
                        e2a = p_eT.tile([128, 2, 512], f8e4, name="e2a", tag="et")
                        nc.scalar.activation(e2a[:], sa[:], AF.Exp, scale=0.125 / 4096.0)
                        if b2 % 2 == 0:
                            # PWL exp on DVE: f8 bits = round(s*EA + EB)
                            e2b = p_eT.tile([128, 2, 512], u8, name="e2b", tag="et")
                            nc.vector.tensor_scalar(e2b[:], sb[:], EA, EB, OP.mult, OP.add)
                            e2b_f8 = e2b[:].bitcast(f8e4)
                        else:
                            e2b = p_eT.tile([128, 2, 512], f8e4, name="e2b", tag="et")
                            nc.scalar.activation(e2b[:], sb[:], AF.Exp, scale=0.125 / 4096.0)
                            e2b_f8 = e2b[:]
                        pend_pv = (o_tl, b2, e2a, e2b_f8, hb)
                flush_pv(pend_pv)
                pend_pv = None
                emit_norm(pend[0])
                pend[0] = None
                # o-proj: psum = (64 w)(64 attn) + 4096*bias -> /4096 + resid
                for half in range(2):
                    ps = psB.tile([128, 2, 512], f32, name="ps_op", tag="B")
                    for m2 in range(2):
                        mt = 2 * half + m2
                        nc.tensor.matmul(ps[:, m2, :],
                                         brow[0:1, (widx // 4) * 512 + mt * 128:
                                              (widx // 4) * 512 + (mt + 1) * 128],
                                         ones_q[:], start=True, stop=False)
                        for h2 in range(2):
                            nc.tensor.matmul(ps[:, m2, :],
                                             w_attn[:, widx + 3, 2 * h2:2 * h2 + 2, mt * 128:(mt + 1) * 128],
                                             Oh_all[:, 2 * h2:2 * h2 + 2, :],
                                             start=False, stop=(h2 == 1), perf_mode=DR)
                    nc.vector.scalar_tensor_tensor(resid_out[:, 2 * half:2 * half + 2, qs],
                                                   ps[:], 1.0 / 4096.0,
                                                   resid_in[:, 2 * half:2 * half + 2, qs],
                                                   OP.mult, OP.add)

        def moe(xn3, X3):
            # X3 row-major staging via DMA crossbar transpose (no PE)
            x3r = sing.tile([128, 8, 512], bf16, name="x3r", tag="x3r")
            for tq in range(8):
                for mt in range(4):
                    nc.sync.dma_start_transpose(x3r[:, tq, mt * 128:(mt + 1) * 128],
                                                X3[:, mt, tq * 128:(tq + 1) * 128])

            # pass 1: hid = SV * (val + bv) * silu(gate + bg), fp8 into arena
            hid_ar = sing.tile([128, 32, 1024], f8e4, name="hid_ar", tag="hid")
            for j in range(32):
                e, jj = j // 16, j % 16
                bv_i = e * 32 + jj
                bg_i = e * 32 + 16 + jj
                g_ps = psA.tile([128, 2, 512], f32, name="g_ps", tag="A")
                for t2 in range(2):
                    for qc in range(2):
                        nc.tensor.matmul(g_ps[:, qc, :],
                                         wi_sb[:, bg_i, 2 * t2:2 * t2 + 2, :],
                                         xn3[:, 2 * t2:2 * t2 + 2, qc * 512:(qc + 1) * 512],
                                         start=(t2 == 0), stop=(t2 == 1), perf_mode=DR)
                sg = p_eT.tile([128, 1024], bf16, name="sg", tag="et")
                nc.scalar.activation(sg[:], g_ps[:], AF.Silu,
                                     bias=bi_sb[:, bg_i:bg_i + 1], scale=1.0 / SI)
                v_ps2 = psB.tile([128, 2, 512], f32, name="v_ps2", tag="B")
                for t2 in range(2):
                    for qc in range(2):
                        nc.tensor.matmul(v_ps2[:, qc, :],
                                         wi_sb[:, bv_i, 2 * t2:2 * t2 + 2, :],
                                         xn3[:, 2 * t2:2 * t2 + 2, qc * 512:(qc + 1) * 512],
                                         start=(t2 == 0), stop=(t2 == 1), perf_mode=DR)
                nc.vector.scalar_tensor_tensor(hid_ar[:, j, :], v_ps2[:],
                                               bi_sb[:, bv_i:bv_i + 1],
                                               sg[:], OP.add, OP.mult)

            # pass 2: token-major out-proj: eo[tok, d] = sum_he hid[he, tok]*wo[he, d]
            # all 8 token blocks accumulate together so each wo chunk loads once
            out_r = out_h.rearrange("(t p) d -> p t d", p=128)
            es_t = [psA.tile([128, 2, 512], f32, name="es_a0", tag="A"),
                    psA.tile([128, 2, 512], f32, name="es_a1", tag="A"),
                    psB.tile([128, 2, 512], f32, name="es_b0", tag="B"),
                    psB.tile([128, 2, 512], f32, name="es_b1", tag="B")]
            eslice = [es_t[i // 2][:, i % 2, :] for i in range(8)]
            for blk in range(8):
                nc.tensor.matmul(eslice[blk], borow[:, 0:128], ones_q[:, 0:512],
                                 start=True, stop=False)
            for jp in range(16):
                wo_t = p_wo.tile([128, 2, 512], f8e4, name="wo_t", tag="wo")
                dma_s(out=wo_t[:], in_=wo_h[jp, :, :].rearrange("p (t m) -> p t m", t=2))
                for blk in range(8):
                    nc.tensor.matmul(eslice[blk],
                                     hid_ar[:, 2 * jp:2 * jp + 2, blk * 128:(blk + 1) * 128],
                                     wo_t[:], start=False, stop=(jp == 15),
                                     perf_mode=DR)
            for pr in range(4):
                tq = 2 * pr
                orm = p_orm.tile([128, 2, 512], f32, name="orm", tag="orm")
                nc.vector.scalar_tensor_tensor(orm[:], es_t[pr][:], 1.0 / (SV * SO),
                                               x3r[:, tq:tq + 2, :], OP.mult, OP.add)
                dma_s(out=out_r[:, tq:tq + 2, :], in_=orm[:])

        xn1 = p_xn.tile([128, 4, 1024], f8e4, name="xn1", tag="xn")
        layer_norm(X_T, xn1)
        X2 = p_resid.tile([128, 4, 1024], bf16, name="X2", tag="resid")
        attention(0, xn1, scene_T, 512, X_T, X2)
        xn2 = p_xn.tile([128, 4, 1024], f8e4, name="xn2", tag="xn")
        layer_norm(X2, xn2)
        X3 = p_resid.tile([128, 4, 1024], bf16, name="X3", tag="resid")
        attention(4, xn2, xn2, 1024, X2, X3)
        xn3 = p_xn.tile([128, 4, 1024], f8e4, name="xn3", tag="xn")
        layer_norm(X3, xn3)
        moe(xn3, X3)

    _legalize_waits(nc)
    _NC = nc
    return nc


def _legalize_waits(nc):
    # Matmult/Ldweights/DMA encodings hold a single sem wait; split extras
    # onto EventSemaphore instructions on the same queue.
    from concourse import mybir
    n = 0
    for fn in nc.m.functions:
        for blk in fn.blocks:
            out = []
            for inst in blk.instructions:
                si = getattr(inst, "sync_info", None)
                ow = list(si.on_wait) if si is not None else []
                if len(ow) > 1 and getattr(inst, "opcode", None) is not None:
                    for j, w in enumerate(ow[:-1]):
                        out.append(mybir.InstEventSemaphore(
                            name=f"{inst.name}-wx{j}",
                            engine=inst.engine,
                            sync_info=mybir.SyncInfo(on_wait=[w], on_update=[]),
                        ))
                        n += 1
                    inst.sync_info = mybir.SyncInfo(
                        on_wait=[ow[-1]], on_update=list(si.on_update))
                out.append(inst)
            blk.instructions = out
    return n


def _silu(v):
    return v / (1.0 + np.exp(-v))


def _softmax(v):
    m = v.max(axis=-1, keepdims=True)
    ex = np.exp(v - m)
    return ex / ex.sum(axis=-1, keepdims=True)


def _f8(x):
    return np.clip(x, -240.0, 240.0).astype(F8)


def _prepare(inputs):
    inp = {k: np.asarray(v, dtype=np.float32) for k, v in inputs.items()}
    x = inp["x"]
    scene = inp["scene_tokens"]
    t = inp["t"]
    g = inp["scene_norm_g"]
    bvec = inp["scene_norm_b"]

    half = D // 2
    freqs = np.exp(-math.log(10000.0) * np.arange(half, dtype=np.float32) / (half - 1)).astype(np.float32)
    ang = t[:, None] * freqs[None, :]
    temb = np.concatenate([np.cos(ang), np.sin(ang)], axis=-1).astype(np.float32)
    ncv = _silu(temb @ inp["ne_w1"] + inp["ne_b1"]) @ inp["ne_w2"] + inp["ne_b2"]

    mod1 = ncv @ inp["ncsa_mod_w"] + inp["ncsa_mod_b"]
    shift1, scale1 = mod1[:, :D], mod1[:, D:]
    mod2 = ncv @ inp["moe_mod_w"] + inp["moe_mod_b"]
    shift2, scale2 = mod2[:, :D], mod2[:, D:]

    probs = _softmax(ncv @ inp["router_w"])
    ti = np.argsort(-probs, axis=-1, kind="stable")[:, :K]
    tw = np.take_along_axis(probs, ti, axis=-1)
    tw = tw / np.clip(tw.sum(-1, keepdims=True), 1e-8, None)

    ca_wq_e = g[:, None] * inp["ca_wq"]
    ca_bq_e = inp["ca_bq"] + bvec @ inp["ca_wq"]
    ca_bo_e = inp["ca_bo"] + inp["ca_bv"] @ inp["ca_wo"]
    zero = np.zeros(D, np.float32)
    vperm = np.concatenate([np.arange(h * HD, (h + 1) * HD) for h in (0, 2, 4, 6, 1, 3, 5, 7)])

    in_maps = []
    for b in range(B):
        s1 = 1.0 + scale1[b]
        sa_wq_e = s1[:, None] * inp["sa_wq"]
        sa_bq_e = inp["sa_bq"] + shift1[b] @ inp["sa_wq"]
        sa_wk_e = s1[:, None] * inp["sa_wk"]
        sa_bk_e = inp["sa_bk"] + shift1[b] @ inp["sa_wk"]
        sa_wv_e = s1[:, None] * inp["sa_wv"]
        sa_bv_e = inp["sa_bv"] + shift1[b] @ inp["sa_wv"]
        sa_bo_e = inp["sa_bo"] + sa_bv_e @ inp["sa_wo"]

        wattn = _f8(SW * np.stack([ca_wq_e, inp["ca_wk"], inp["ca_wv"][:, vperm], inp["ca_wo"],
                                   sa_wq_e, sa_wk_e, sa_wv_e[:, vperm], inp["sa_wo"]]))
        # Q/K biases pre-scaled by SW (psum is 64x); V zero; O via brow
        battn = SW * np.stack([ca_bq_e, inp["ca_bk"], zero, zero,
                               sa_bq_e, sa_bk_e, zero, zero]).astype(np.float32)
        brow_v = np.zeros((1, 1024), np.float32)
        brow_v[0, 0:512] = SW * SW * ca_bo_e
        brow_v[0, 512:1024] = SW * SW * sa_bo_e

        s2 = 1.0 + scale2[b]
        Wis, bis, Wos = [], [], []
        bo_moe = np.zeros(D, np.float32)
        for k in range(K):
            eidx = int(ti[b, k])
            w = np.float32(tw[b, k])
            Wi_e = inp["fc_in_w"][eidx]
            Wis.append(s2[:, None] * Wi_e)
            bis.append(inp["fc_in_b"][eidx] + shift2[b] @ Wi_e)
            Wos.append(w * inp["fc_out_w"][eidx])
            bo_moe = bo_moe + w * inp["fc_out_b"][eidx]
        Wi_cat = np.concatenate(Wis, axis=1)          # [D, 2*2HE] = [512, 8192]
        bi_cat = np.concatenate(bis, axis=0)          # [8192]
        Wo_cat = np.concatenate(Wos, axis=0)          # [2*HE, D] = [4096, 512]

        # wi chunks [64][d_lo 128][d_hi 4 * h_lo 128]; val chunks x SV, gate x SI
        wi_pt = np.ascontiguousarray(
            Wi_cat.reshape(4, 128, 64, 128).transpose(2, 1, 0, 3).reshape(64, 128, 512))
        wsc = np.zeros((64, 1, 1), np.float32)
        bsc = np.zeros(64, np.float32)
        for e in range(2):
            wsc[e * 32:e * 32 + 16] = SV
            wsc[e * 32 + 16:e * 32 + 32] = SI
            bsc[e * 32:e * 32 + 16] = SV
            bsc[e * 32 + 16:e * 32 + 32] = 1.0
        wi8 = _f8(wi_pt * wsc)
        bi_pt = np.ascontiguousarray(bi_cat.reshape(64, 128) * bsc[:, None]).astype(np.float32)
        # wo chunks [16][he_lo 128][he_hi 2 * d 512], x SO
        wo8 = _f8(SO * np.ascontiguousarray(
            Wo_cat.reshape(16, 2, 128, 512).transpose(0, 2, 1, 3).reshape(16, 128, 1024)))
        borow_v = (SV * SO * bo_moe).reshape(1, 512)

        in_maps.append({
            "x": np.ascontiguousarray(x[b]).astype(BF16),
            "scene": np.ascontiguousarray(scene[b]).astype(BF16),
            "wattn": np.ascontiguousarray(wattn),
            "battn": np.ascontiguousarray(battn),
            "brow": brow_v.astype(BF16),
            "wi": wi8,
            "bi_t": bi_pt,
            "wo": wo8,
            "borow": borow_v.astype(BF16),
        })
    return in_maps


def _run(in_maps, trace=False):
    from concourse.bass_utils import run_bass_kernel_spmd
    nc = _build()
    return run_bass_kernel_spmd(nc, in_maps, list(range(NCORES)), trace=trace)


def kernel(**inputs):
    in_maps = _prepare(inputs)
    res = _run(in_maps)
    return np.stack([np.asarray(res.results[i]["out"], dtype=np.float32) for i in range(B)])


# revision 18
# speedup vs baseline: 1.0392x; 1.0392x over previous
import math
import sys

for _p in ("/root/.axon_site", "/root/.axon_site/_ro/trn_rl_repo", "/opt/trn_rl_repo"):
    if _p not in sys.path:
        sys.path.append(_p)

import numpy as np
import ml_dtypes

BF16 = ml_dtypes.bfloat16
F8 = ml_dtypes.float8_e4m3  # IEEE-style e4m3: max 240 == TRN FP8_EXP4

B, L, NS = 8, 1024, 512
D, NH, DN = 512, 8, 256
E, K, HE = 8, 2, 2048
HD = D // NH
EPS = 1e-5
NCORES = 8

SW = 64.0    # attention weight fp8 scale
SI = 64.0    # moe gate fc_in scale
SV = 32.0    # moe val fc_in scale
SO = 64.0    # moe fc_out scale

EA = math.log2(math.e) * 0.125 / 4096.0 * 8.0
EB = 56.0 - 0.344

_NC = None


def _build():
    global _NC
    if _NC is not None:
        return _NC
    from concourse import bass, tile, mybir, masks

    f32 = mybir.dt.float32
    bf16 = mybir.dt.bfloat16
    f8e4 = mybir.dt.float8e4
    u8 = mybir.dt.uint8
    AF = mybir.ActivationFunctionType
    OP = mybir.AluOpType
    DR = mybir.MatmulPerfMode.DoubleRow

    nc = bass.Bass()
    x_h = nc.declare_dram_parameter("x", [L, D], bf16, isOutput=False)
    scene_h = nc.declare_dram_parameter("scene", [NS, D], bf16, isOutput=False)
    wattn_h = nc.declare_dram_parameter("wattn", [8, D, D], f8e4, isOutput=False)
    battn_h = nc.declare_dram_parameter("battn", [8, D], f32, isOutput=False)
    brow_h = nc.declare_dram_parameter("brow", [1, 1024], bf16, isOutput=False)
    wi_h = nc.declare_dram_parameter("wi", [64, 128, 512], f8e4, isOutput=False)
    bi_h = nc.declare_dram_parameter("bi_t", [64, 128], f32, isOutput=False)
    wo_h = nc.declare_dram_parameter("wo", [16, 128, 1024], f8e4, isOutput=False)
    borow_h = nc.declare_dram_parameter("borow", [1, 512], bf16, isOutput=False)
    out_h = nc.declare_dram_parameter("out", [L, D], f32, isOutput=True)

    with tile.TileContext(nc) as tc, \
         tc.tile_pool(name="sing", bufs=1) as sing, \
         tc.tile_pool(name="p_resid", bufs=2) as p_resid, \
         tc.tile_pool(name="p_xn", bufs=2) as p_xn, \
         tc.tile_pool(name="p_qkv", bufs=1) as p_qkv, \
         tc.tile_pool(name="p_oh", bufs=2) as p_oh, \
         tc.tile_pool(name="p_eT", bufs=4) as p_eT, \
         tc.tile_pool(name="p_ms", bufs=6) as p_ms, \
         tc.tile_pool(name="p_msx", bufs=4) as p_msx, \
         tc.tile_pool(name="p_xb", bufs=2) as p_xb, \
         tc.tile_pool(name="p_wo", bufs=4) as p_wo, \
         tc.tile_pool(name="p_orm", bufs=2) as p_orm, \
         tc.tile_pool(name="p_nt", bufs=2) as p_nt, \
         tc.tile_pool(name="psA", bufs=2, space="PSUM") as psA, \
         tc.tile_pool(name="psB", bufs=2, space="PSUM") as psB:

        dma_s = nc.sync.dma_start
        dma_a = nc.scalar.dma_start

        ident = sing.tile([128, 128], f32, name="ident", tag="ident")
        masks.make_identity(nc, ident[:])
        ones_sq = sing.tile([128, 64], bf16, name="ones_sq", tag="ones_sq")
        nc.vector.memset(ones_sq[:], 1.0)
        ones_q = sing.tile([1, 512], bf16, name="ones_q", tag="ones_qq")
        nc.vector.memset(ones_q[:], 1.0)
        ones_big = sing.tile([128, 512], bf16, name="ones_big", tag="ones_b")
        nc.vector.memset(ones_big[:], 1.0)
        ones_bc = sing.tile([128, 128], bf16, name="ones_bc", tag="ones_bc")
        nc.vector.memset(ones_bc[:], 1.0)

        # HAM warm-up spins + ln/exp table preload during the DMA window
        dummy = p_ms.tile([1, 1], f32, name="dummy", tag="ms")
        nc.scalar.activation(dummy[:], ident[0:1, 0:1], AF.Ln)
        dummy2 = p_ms.tile([1, 1], f32, name="dummy2", tag="ms")
        nc.scalar.activation(dummy2[:], ident[0:1, 0:1], AF.Exp)

        # x -> feature-major bf16 spine directly via DMA crossbar transpose
        X_T = p_resid.tile([128, 4, 1024], bf16, name="X_T", tag="resid")
        for mt in range(4):
            nc.sync.dma_start_transpose(X_T[:, mt, :], x_h[:, mt * 128:(mt + 1) * 128])
        scene_Tb = p_xb.tile([128, 4, 512], bf16, name="scene_Tb", tag="xbsq")
        for mt in range(4):
            nc.sync.dma_start_transpose(scene_Tb[:, mt, :], scene_h[:, mt * 128:(mt + 1) * 128])
        w_attn = sing.tile([128, 8, 4, 512], f8e4, name="w_attn", tag="w_attn")
        dma_s(out=w_attn[:], in_=wattn_h.rearrange("i (t p) d -> p i t d", p=128))
        b_attn = sing.tile([128, 8, 4], f32, name="b_attn", tag="b_attn")
        dma_s(out=b_attn[:], in_=battn_h.rearrange("i (t p) -> p i t", p=128))
        brow = sing.tile([1, 1024], bf16, name="brow", tag="brow")
        dma_s(out=brow[:], in_=brow_h[:, :])
        borow = sing.tile([1, 512], bf16, name="borow", tag="borow")
        dma_s(out=borow[:], in_=borow_h[:, :])
        bi_sb = sing.tile([128, 64], f32, name="bi_sb", tag="bi_sb")
        dma_s(out=bi_sb[:], in_=bi_h.rearrange("b p -> p b"))

        # full moe fc_in weight prefetch (trickles in during CA/SA)
        wi_sb = sing.tile([128, 64, 4, 128], f8e4, name="wi_sb", tag="wi_sb")
        for i in range(64):
            dma_s(out=wi_sb[:, i], in_=wi_h[i, :, :].rearrange("p (t m) -> p t m", t=4))

        for _w in range(30):
            spin = psA.tile([128, 2, 512], f32, name="spin", tag="A")
            nc.tensor.matmul(spin[0:64, 0, :], ones_sq[:, :], ones_big[:],
                             start=True, stop=True)

        scene_T = sing.tile([128, 4, 512], f8e4, name="scene_T", tag="scene_T")
        nc.vector.tensor_scalar_mul(scene_T[:], scene_Tb[:], 1.0)

        # V arena: even heads at 0:64 (+den ones col 64), odd heads at
        # 144:208 (+den ones col 80, zeros 81:144); constant regions set once
        V2 = p_qkv.tile([128, 8, 4, 208], f8e4, name="V2", tag="ve")
        nc.vector.memset(V2[:, :, :, 64:65], 1.0)
        nc.vector.memset(V2[:, :, :, 80:81], 1.0)
        nc.vector.memset(V2[:, :, :, 81:144], 0.0)

        def ln_spins(n):
            for _s in range(n):
                spin = psA.tile([128, 2, 512], f32, name="spin", tag="A")
                nc.tensor.matmul(spin[0:64, 0, :], ones_sq[:, :], ones_big[:],
                                 start=True, stop=True)

        def layer_norm(src, xn):
            ln_spins(3)
            for qc in range(2):
                qs = slice(qc * 512, (qc + 1) * 512)
                sq = p_xb.tile([128, 4, 512], bf16, name="sq", tag="xbsq")
                nc.vector.tensor_tensor(sq[:], src[:, :, qs], src[:, :, qs], OP.mult)
                st = psA.tile([128, 2, 512], f32, name="st", tag="A")
                for kt in range(4):
                    nc.tensor.matmul(st[:, 0, :], ones_bc[:], src[:, kt, qs],
                                     start=(kt == 0), stop=(kt == 3))
                for kt in range(4):
                    nc.tensor.matmul(st[:, 1, :], ones_bc[:], sq[:, kt, :],
                                     start=(kt == 0), stop=(kt == 3))
                m_bf = p_ms.tile([128, 512], bf16, name="m_bf", tag="ms")
                nc.vector.tensor_scalar_mul(m_bf[:], st[:, 0, :], 1.0 / 512.0)
                e2v = p_ms.tile([128, 512], f32, name="e2v", tag="ms")
                nc.vector.tensor_scalar(e2v[:], st[:, 1, :], 1.0 / 512.0, EPS,
                                        OP.mult, OP.add)
                mm = p_ms.tile([128, 512], bf16, name="mm", tag="ms")
                nc.vector.tensor_tensor(mm[:], m_bf[:], m_bf[:], OP.mult)
                var = p_ms.tile([128, 512], f32, name="var", tag="ms")
                nc.vector.tensor_tensor(var[:], e2v[:], mm[:], OP.subtract)
                # 1/sqrt(var) = exp(-0.5 ln var): stays in the ln/exp table set
                lnv = p_ms.tile([128, 512], f32, name="lnv", tag="ms")
                nc.scalar.activation(lnv[:], var[:], AF.Ln)
                rsq = p_ms.tile([128, 512], bf16, name="rsq", tag="ms")
                nc.scalar.activation(rsq[:], lnv[:], AF.Exp, scale=-0.5)
                for mt in range(4):
                    xs = p_msx.tile([128, 512], bf16, name="xs", tag="msx")
                    nc.vector.tensor_tensor(xs[:], src[:, mt, qs], m_bf[:], OP.subtract)
                    nc.vector.tensor_tensor(xn[:, mt, qs], xs[:], rsq[:], OP.mult)
            ln_spins(3)

        def attention(widx, xq_T, kv_T, kv_len, resid_in, resid_out):
            nkp = kv_len // 128
            nkc = kv_len // 512
            nb2 = nkp // 2
            # fp8 DoubleRow projections: psum = 64*W @ x; Scalar drains Q
            Q_T = p_qkv.tile([128, 4, 1024], bf16, name="Q_T", tag="q")
            K_T = p_qkv.tile([128, 4, 1024], bf16, name="K_T", tag="k")
            for mt in range(4):
                q_ps = psA.tile([128, 2, 512], f32, name="q_ps", tag="A")
                for t2 in range(2):
                    for qc in range(2):
                        nc.tensor.matmul(q_ps[:, qc, :],
                                         w_attn[:, widx, 2 * t2:2 * t2 + 2, mt * 128:(mt + 1) * 128],
                                         xq_T[:, 2 * t2:2 * t2 + 2, qc * 512:(qc + 1) * 512],
                                         start=(t2 == 0), stop=(t2 == 1), perf_mode=DR)
                k_ps = psB.tile([128, 2, 512], f32, name="k_ps", tag="B")
                for t2 in range(2):
                    for kc in range(nkc):
                        nc.tensor.matmul(k_ps[:, kc, :],
                                         w_attn[:, widx + 1, 2 * t2:2 * t2 + 2, mt * 128:(mt + 1) * 128],
                                         kv_T[:, 2 * t2:2 * t2 + 2, kc * 512:(kc + 1) * 512],
                                         start=(t2 == 0), stop=(t2 == 1), perf_mode=DR)
                nc.vector.tensor_scalar_add(Q_T[:, mt, :], q_ps[:],
                                            b_attn[:, widx, mt:mt + 1])
                nc.vector.tensor_scalar_add(K_T[:, mt, 0:kv_len],
                                            k_ps[:, 0:nkc, :],
                                            b_attn[:, widx + 1, mt:mt + 1])
            for kp in range(nkp):
                v_ps = psA.tile([128, 2, 4, 64], f32, name="v_ps", tag="A")
                for t2 in range(2):
                    nc.tensor.matmul(v_ps[:],
                                     kv_T[:, 2 * t2:2 * t2 + 2, kp * 128:(kp + 1) * 128],
                                     w_attn[:, widx + 2, 2 * t2:2 * t2 + 2, :],
                                     start=(t2 == 0), stop=(t2 == 1), perf_mode=DR)
                nc.vector.tensor_scalar_mul(V2[:, kp, :, 0:64], v_ps[:, 0, :, :], 1.0)
                nc.vector.tensor_scalar_mul(V2[:, kp, :, 144:208], v_ps[:, 1, :, :], 1.0)

            def emit_norm(st):
                o_tl, rcb, hb, Oh_all, fast = st
                rb = psA.tile([128, 2, 512], f32, name="rb", tag="A")
                nc.tensor.matmul(rb[0:64, 0, :], ones_sq[64:65, 0:64], rcb[64:65, :],
                                 start=True, stop=True)
                nc.tensor.matmul(rb[64:128, 0, :], ones_sq[0:1, 0:64], rcb[0:1, :],
                                 start=True, stop=True)
                rb_sb = p_eT.tile([128, 512], bf16, name="rb_sb", tag="rbs")
                if fast:
                    # rcb already holds 1/den (Scalar path): broadcast only
                    nc.vector.tensor_scalar_mul(rb_sb[:], rb[:, 0, :], 1.0)
                else:
                    # rcb holds den: 1/den via Newton on idle GpSimd
                    den_sb = p_nt.tile([128, 512], f32, name="den_sb", tag="dsb")
                    nc.vector.tensor_scalar_mul(den_sb[:], rb[:, 0, :], 1.0)
                    z0 = 1.0 / float(kv_len)
                    y1 = p_nt.tile([128, 512], f32, name="y1", tag="y1", bufs=1)
                    nc.gpsimd.tensor_scalar(y1[:], den_sb[:], -z0 * z0, 2.0 * z0,
                                            OP.mult, OP.add)
                    tn = p_nt.tile([128, 512], f32, name="tn", tag="tn", bufs=1)
                    nc.gpsimd.tensor_tensor(tn[:], den_sb[:], y1[:], OP.mult)
                    un = p_nt.tile([128, 512], f32, name="un", tag="un", bufs=1)
                    nc.gpsimd.tensor_scalar(un[:], tn[:], -1.0, 2.0, OP.mult, OP.add)
                    nc.gpsimd.tensor_tensor(rb_sb[:], y1[:], un[:], OP.mult)
                nc.vector.tensor_tensor(Oh_all[0:64, hb, :], o_tl[0:64, 0, :],
                                        rb_sb[0:64, :], OP.mult)
                nc.vector.tensor_tensor(Oh_all[64:128, hb, :], o_tl[64:128, 1, :],
                                        rb_sb[64:128, :], OP.mult)

            for qc in range(2):
                qs = slice(qc * 512, (qc + 1) * 512)
                # Oh_all = 64 * attn_out per head, fp8
                Oh_all = p_oh.tile([128, 4, 512], f8e4, name="Oh_all", tag="oh")
                pend = [None]

                def emit_den(o_tl, hb):
                    # denominator rows -> SBUF; queue Oh-normalize one hb late
                    rcb = p_ms.tile([65, 512], bf16, name="rcb", tag="ms")
                    if hb == 3:
                        # last head: no later work hides the Newton chain, so
                        # take the Scalar ln/exp path (Scalar idles here)
                        lnd = p_ms.tile([65, 512], f32, name="lnd", tag="ms")
                        nc.scalar.activation(lnd[64:65, :], o_tl[64:65, 0, :], AF.Ln)
                        nc.scalar.activation(lnd[0:1, :], o_tl[0:1, 1, :], AF.Ln)
                        nc.scalar.activation(rcb[64:65, :], lnd[64:65, :], AF.Exp, scale=-1.0)
                        nc.scalar.activation(rcb[0:1, :], lnd[0:1, :], AF.Exp, scale=-1.0)
                    else:
                        nc.vector.tensor_scalar_mul(rcb[64:65, :], o_tl[64:65, 0, :], 1.0)
                        nc.vector.tensor_scalar_mul(rcb[0:1, :], o_tl[0:1, 1, :], 1.0)
                    pend[0] = (o_tl, rcb, hb, Oh_all, hb == 3)

                def flush_pv(st):
                    # software-pipelined PV: issued after the NEXT step's
                    # scores so the Tensor queue never stalls behind exp
                    o_tl, b2, e2a, e2b_f8, hb = st
                    nc.tensor.matmul(o_tl[0:65, 0, :], V2[:, 2 * b2:2 * b2 + 2, hb, 0:65],
                                     e2a[:], start=(b2 == 0), stop=(b2 == nb2 - 1),
                                     perf_mode=DR)
                    nc.tensor.matmul(o_tl[:, 1, :], V2[:, 2 * b2:2 * b2 + 2, hb, 80:208],
                                     e2b_f8, start=(b2 == 0), stop=(b2 == nb2 - 1),
                                     perf_mode=DR)
                    if b2 == 0 and pend[0] is not None:
                        # previous head's normalize: start its Newton chain now
                        # so it hides under this whole head's steps
                        emit_norm(pend[0])
                        pend[0] = None
                    if b2 == nb2 - 1:
                        emit_den(o_tl, hb)

                pend_pv = None
                for hb in range(4):
                    o_tl = psB.tile([128, 2, 512], f32, name="o_tl", tag="B")
                    for b2 in range(nb2):
                        sa = psA.tile([128, 2, 512], f32, name="sa", tag="A")
                        sb = psA.tile([128, 2, 512], f32, name="sb", tag="A")
                        # row-packed score pairs: both heads concurrently
                        for k2 in range(2):
                            kp = 2 * b2 + k2
                            nc.tensor.matmul(sa[:, k2, :],
                                             K_T[0:64, hb, kp * 128:(kp + 1) * 128],
                                             Q_T[0:64, hb, qs],
                                             start=True, stop=True)
                            nc.tensor.matmul(sb[:, k2, :],
                                             K_T[64:128, hb, kp * 128:(kp + 1) * 128],
                                             Q_T[64:128, hb, qs],
                                             start=True, stop=True)
                        if pend_pv is not None:
                            flush_pv(pend_pv)
                        e2a = p_eT.tile([128, 2, 512], f8e4, name="e2a", tag="et")
                        nc.scalar.activation(e2a[:], sa[:], AF.Exp, scale=0.125 / 4096.0)
                        if b2 % 2 == 0:
                            # PWL exp on DVE: f8 bits = round(s*EA + EB)
                            e2b = p_eT.tile([128, 2, 512], u8, name="e2b", tag="et")
                            nc.vector.tensor_scalar(e2b[:], sb[:], EA, EB, OP.mult, OP.add)
                            e2b_f8 = e2b[:].bitcast(f8e4)
                        else:
                            e2b = p_eT.tile([128, 2, 512], f8e4, name="e2b", tag="et")
                            nc.scalar.activation(e2b[:], sb[:], AF.Exp, scale=0.125 / 4096.0)
                            e2b_f8 = e2b[:]
                        pend_pv = (o_tl, b2, e2a, e2b_f8, hb)
                flush_pv(pend_pv)
                pend_pv = None
                emit_norm(pend[0])
                pend[0] = None
                # o-proj: psum = (64 w)(64 attn) + 4096*bias -> /4096 + resid
                for half in range(2):
                    ps = psB.tile([128, 2, 512], f32, name="ps_op", tag="B")
                    for m2 in range(2):
                        mt = 2 * half + m2
                        nc.tensor.matmul(ps[:, m2, :],
                                         brow[0:1, (widx // 4) * 512 + mt * 128:
                                              (widx // 4) * 512 + (mt + 1) * 128],
                                         ones_q[:], start=True, stop=False)
                        for h2 in range(2):
                            nc.tensor.matmul(ps[:, m2, :],
                                             w_attn[:, widx + 3, 2 * h2:2 * h2 + 2, mt * 128:(mt + 1) * 128],
                                             Oh_all[:, 2 * h2:2 * h2 + 2, :],
                                             start=False, stop=(h2 == 1), perf_mode=DR)
                    nc.vector.scalar_tensor_tensor(resid_out[:, 2 * half:2 * half + 2, qs],
                                                   ps[:], 1.0 / 4096.0,
                                                   resid_in[:, 2 * half:2 * half + 2, qs],
                                                   OP.mult, OP.add)

        def moe(xn3, X3):
            # X3 row-major staging via DMA crossbar transpose (no PE)
            x3r = sing.tile([128, 8, 512], bf16, name="x3r", tag="x3r")
            for tq in range(8):
                for mt in range(4):
                    nc.sync.dma_start_transpose(x3r[:, tq, mt * 128:(mt + 1) * 128],
                                                X3[:, mt, tq * 128:(tq + 1) * 128])

            # pass 1: hid = SV * (val + bv) * silu(gate + bg), fp8 into arena
            hid_ar = sing.tile([128, 32, 1024], f8e4, name="hid_ar", tag="hid")
            for j in range(32):
                e, jj = j // 16, j % 16
                bv_i = e * 32 + jj
                bg_i = e * 32 + 16 + jj
                g_ps = psA.tile([128, 2, 512], f32, name="g_ps", tag="A")
                for t2 in range(2):
                    for qc in range(2):
                        nc.tensor.matmul(g_ps[:, qc, :],
                                         wi_sb[:, bg_i, 2 * t2:2 * t2 + 2, :],
                                         xn3[:, 2 * t2:2 * t2 + 2, qc * 512:(qc + 1) * 512],
                                         start=(t2 == 0), stop=(t2 == 1), perf_mode=DR)
                sg = p_eT.tile([128, 1024], bf16, name="sg", tag="et")
                nc.scalar.activation(sg[:], g_ps[:], AF.Silu,
                                     bias=bi_sb[:, bg_i:bg_i + 1], scale=1.0 / SI)
                v_ps2 = psB.tile([128, 2, 512], f32, name="v_ps2", tag="B")
                for t2 in range(2):
                    for qc in range(2):
                        nc.tensor.matmul(v_ps2[:, qc, :],
                                         wi_sb[:, bv_i, 2 * t2:2 * t2 + 2, :],
                                         xn3[:, 2 * t2:2 * t2 + 2, qc * 512:(qc + 1) * 512],
                                         start=(t2 == 0), stop=(t2 == 1), perf_mode=DR)
                nc.vector.scalar_tensor_tensor(hid_ar[:, j, :], v_ps2[:],
                                               bi_sb[:, bv_i:bv_i + 1],
                                               sg[:], OP.add, OP.mult)

            # pass 2: token-major out-proj: eo[tok, d] = sum_he hid[he, tok]*wo[he, d]
            # all 8 token blocks accumulate together so each wo chunk loads once
            out_r = out_h.rearrange("(t p) d -> p t d", p=128)
            es_t = [psA.tile([128, 2, 512], f32, name="es_a0", tag="A"),
                    psA.tile([128, 2, 512], f32, name="es_a1", tag="A"),
                    psB.tile([128, 2, 512], f32, name="es_b0", tag="B"),
                    psB.tile([128, 2, 512], f32, name="es_b1", tag="B")]
            eslice = [es_t[i // 2][:, i % 2, :] for i in range(8)]
            for blk in range(8):
                nc.tensor.matmul(eslice[blk], borow[:, 0:128], ones_q[:, 0:512],
                                 start=True, stop=False)
            for jp in range(16):
                wo_t = p_wo.tile([128, 2, 512], f8e4, name="wo_t", tag="wo")
                dma_s(out=wo_t[:], in_=wo_h[jp, :, :].rearrange("p (t m) -> p t m", t=2))
                for blk in range(8):
                    nc.tensor.matmul(eslice[blk],
                                     hid_ar[:, 2 * jp:2 * jp + 2, blk * 128:(blk + 1) * 128],
                                     wo_t[:], start=False, stop=(jp == 15),
                                     perf_mode=DR)
            for pr in range(4):
                tq = 2 * pr
                orm = p_orm.tile([128, 2, 512], f32, name="orm", tag="orm")
                nc.vector.scalar_tensor_tensor(orm[:], es_t[pr][:], 1.0 / (SV * SO),
                                               x3r[:, tq:tq + 2, :], OP.mult, OP.add)
                dma_s(out=out_r[:, tq:tq + 2, :], in_=orm[:])

        xn1 = p_xn.tile([128, 4, 1024], f8e4, name="xn1", tag="xn")
        layer_norm(X_T, xn1)
        X2 = p_resid.tile([128, 4, 1024], bf16, name="X2", tag="resid")
        attention(0, xn1, scene_T, 512, X_T, X2)
        xn2 = p_xn.tile([128, 4, 1024], f8e4, name="xn2", tag="xn")
        layer_norm(X2, xn2)
        X3 = p_resid.tile([128, 4, 1024], bf16, name="X3", tag="resid")
        attention(4, xn2, xn2, 1024, X2, X3)
        xn3 = p_xn.tile([128, 4, 1024], f8e4, name="xn3", tag="xn")
        layer_norm(X3, xn3)
        moe(xn3, X3)

    _legalize_waits(nc)
    _NC = nc
    return nc


def _legalize_waits(nc):
    # Matmult/Ldweights/DMA encodings hold a single sem wait; split extras
    # onto EventSemaphore instructions on the same queue.
    from concourse import mybir
    n = 0
    for fn in nc.m.functions:
        for blk in fn.blocks:
            out = []
            for inst in blk.instructions:
                si = getattr(inst, "sync_info", None)
                ow = list(si.on_wait) if si is not None else []
                if len(ow) > 1 and getattr(inst, "opcode", None) is not None:
                    for j, w in enumerate(ow[:-1]):
                        out.append(mybir.InstEventSemaphore(
                            name=f"{inst.name}-wx{j}",
                            engine=inst.engine,
                            sync_info=mybir.SyncInfo(on_wait=[w], on_update=[]),
                        ))
                        n += 1
                    inst.sync_info = mybir.SyncInfo(
                        on_wait=[ow[-1]], on_update=list(si.on_update))
                out.append(inst)
            blk.instructions = out
    return n


def _silu(v):
    return v / (1.0 + np.exp(-v))


def _softmax(v):
    m = v.max(axis=-1, keepdims=True)
    ex = np.exp(v - m)
    return ex / ex.sum(axis=-1, keepdims=True)


def _f8(x):
    return np.clip(x, -240.0, 240.0).astype(F8)


def _prepare(inputs):
    inp = {k: np.asarray(v, dtype=np.float32) for k, v in inputs.items()}
    x = inp["x"]
    scene = inp["scene_tokens"]
    t = inp["t"]
    g = inp["scene_norm_g"]
    bvec = inp["scene_norm_b"]

    half = D // 2
    freqs = np.exp(-math.log(10000.0) * np.arange(half, dtype=np.float32) / (half - 1)).astype(np.float32)
    ang = t[:, None] * freqs[None, :]
    temb = np.concatenate([np.cos(ang), np.sin(ang)], axis=-1).astype(np.float32)
    ncv = _silu(temb @ inp["ne_w1"] + inp["ne_b1"]) @ inp["ne_w2"] + inp["ne_b2"]

    mod1 = ncv @ inp["ncsa_mod_w"] + inp["ncsa_mod_b"]
    shift1, scale1 = mod1[:, :D], mod1[:, D:]
    mod2 = ncv @ inp["moe_mod_w"] + inp["moe_mod_b"]
    shift2, scale2 = mod2[:, :D], mod2[:, D:]

    probs = _softmax(ncv @ inp["router_w"])
    ti = np.argsort(-probs, axis=-1, kind="stable")[:, :K]
    tw = np.take_along_axis(probs, ti, axis=-1)
    tw = tw / np.clip(tw.sum(-1, keepdims=True), 1e-8, None)

    ca_wq_e = g[:, None] * inp["ca_wq"]
    ca_bq_e = inp["ca_bq"] + bvec @ inp["ca_wq"]
    ca_bo_e = inp["ca_bo"] + inp["ca_bv"] @ inp["ca_wo"]
    zero = np.zeros(D, np.float32)
    vperm = np.concatenate([np.arange(h * HD, (h + 1) * HD) for h in (0, 2, 4, 6, 1, 3, 5, 7)])

    in_maps = []
    for b in range(B):
        s1 = 1.0 + scale1[b]
        sa_wq_e = s1[:, None] * inp["sa_wq"]
        sa_bq_e = inp["sa_bq"] + shift1[b] @ inp["sa_wq"]
        sa_wk_e = s1[:, None] * inp["sa_wk"]
        sa_bk_e = inp["sa_bk"] + shift1[b] @ inp["sa_wk"]
        sa_wv_e = s1[:, None] * inp["sa_wv"]
        sa_bv_e = inp["sa_bv"] + shift1[b] @ inp["sa_wv"]
        sa_bo_e = inp["sa_bo"] + sa_bv_e @ inp["sa_wo"]

        wattn = _f8(SW * np.stack([ca_wq_e, inp["ca_wk"], inp["ca_wv"][:, vperm], inp["ca_wo"],
                                   sa_wq_e, sa_wk_e, sa_wv_e[:, vperm], inp["sa_wo"]]))
        # Q/K biases pre-scaled by SW (psum is 64x); V zero; O via brow
        battn = SW * np.stack([ca_bq_e, inp["ca_bk"], zero, zero,
                               sa_bq_e, sa_bk_e, zero, zero]).astype(np.float32)
        brow_v = np.zeros((1, 1024), np.float32)
        brow_v[0, 0:512] = SW * SW * ca_bo_e
        brow_v[0, 512:1024] = SW * SW * sa_bo_e

        s2 = 1.0 + scale2[b]
        Wis, bis, Wos = [], [], []
        bo_moe = np.zeros(D, np.float32)
        for k in range(K):
            eidx = int(ti[b, k])
            w = np.float32(tw[b, k])
            Wi_e = inp["fc_in_w"][eidx]
            Wis.append(s2[:, None] * Wi_e)
            bis.append(inp["fc_in_b"][eidx] + shift2[b] @ Wi_e)
            Wos.append(w * inp["fc_out_w"][eidx])
            bo_moe = bo_moe + w * inp["fc_out_b"][eidx]
        Wi_cat = np.concatenate(Wis, axis=1)          # [D, 2*2HE] = [512, 8192]
        bi_cat = np.concatenate(bis, axis=0)          # [8192]
        Wo_cat = np.concatenate(Wos, axis=0)          # [2*HE, D] = [4096, 512]

        # wi chunks [64][d_lo 128][d_hi 4 * h_lo 128]; val chunks x SV, gate x SI
        wi_pt = np.ascontiguousarray(
            Wi_cat.reshape(4, 128, 64, 128).transpose(2, 1, 0, 3).reshape(64, 128, 512))
        wsc = np.zeros((64, 1, 1), np.float32)
        bsc = np.zeros(64, np.float32)
        for e in range(2):
            wsc[e * 32:e * 32 + 16] = SV
            wsc[e * 32 + 16:e * 32 + 32] = SI
            bsc[e * 32:e * 32 + 16] = SV
            bsc[e * 32 + 16:e * 32 + 32] = 1.0
        wi8 = _f8(wi_pt * wsc)
        bi_pt = np.ascontiguousarray(bi_cat.reshape(64, 128) * bsc[:, None]).astype(np.float32)
        # wo chunks [16][he_lo 128][he_hi 2 * d 512], x SO
        wo8 = _f8(SO * np.ascontiguousarray(
            Wo_cat.reshape(16, 2, 128, 512).transpose(0, 2, 1, 3).reshape(16, 128, 1024)))
        borow_v = (SV * SO * bo_moe).reshape(1, 512)

        in_maps.append({
            "x": np.ascontiguousarray(x[b]).astype(BF16),
            "scene": np.ascontiguousarray(scene[b]).astype(BF16),
            "wattn": np.ascontiguousarray(wattn),
            "battn": np.ascontiguousarray(battn),
            "brow": brow_v.astype(BF16),
            "wi": wi8,
            "bi_t": bi_pt,
            "wo": wo8,
            "borow": borow_v.astype(BF16),
        })
    return in_maps


def _run(in_maps, trace=False):
    from concourse.bass_utils import run_bass_kernel_spmd
    nc = _build()
    return run_bass_kernel_spmd(nc, in_maps, list(range(NCORES)), trace=trace)


def kernel(**inputs):
    in_maps = _prepare(inputs)
    res = _run(in_maps)
    return np.stack([np.asarray(res.results[i]["out"], dtype=np.float32) for i in range(B)])
